# revision 25
# baseline (speedup 1.0000x reference)
"""Trainium2 Bass kernel for nn_ClementsBellNxN (N=512, 8 cores).

Sharding: column-wise, 64 columns per core; zero communication.

Algorithm (per core, per step i of 256):
  even half-step: fused operator E_k = Mmi@diag(e^{i pa[2k]},e^{i pa[2k+1]})@Mmi
     applied to row pairs (2k, 2k+1); 2x2 symmetric complex [[a,b],[b,d]].
  odd half-step:  same with pb on pairs (2k+1, 2k+2); edge rows 0/511 get pure
     phase rotations, absorbed into spare coefficient lanes.

Storage: pair k -> (partition p=k//2, free-block b=k%2); tiles T(even rows)/
U(odd rows) split into 8 channels [128,64]: {T,U} x {re,im} x {b0,b1}.
The odd half's "odd-k" range needs t_{k+1} = T[p+1, b0]: partition shifts are
illegal in engine APs, so the shift runs on the idle TensorEngine via constant
permutation matmuls (Pfwd/Pbwd), with corner lanes carrying the edge rows.

Per 128-lane half-block the 2x2 apply uses the beta-symmetry trick (m=b*(t+u))
with a runtime-registered custom DVE op CMUL_SUB_ANT (out = C0*Src0 - C1*Src1,
per-partition scalar columns) fusing each complex rotation into one DVE
instruction. Engine split: DVE fused rotations + PSUM-adjacent STT chains,
GPSIMD the tensor-adds, ScalarE the tsh PSUM->SBUF copies, PE the shifts.
Coefficients pack 9 columns per (step, half, range): br,bi,-br, ar,ai,-ar,
dr,di,-dr (a := alpha-beta, d := delta-beta).
"""
import numpy as np

N = 512
S = 256
NCORES = 8
COLS = N // NCORES  # 64
IL = 0.05
IMB = 0.005
_sq = np.sqrt(1.0 - IL)
A = np.float64(np.float32(_sq * np.sqrt(0.5 + IMB)))
B = np.float64(np.float32(_sq * np.sqrt(0.5 - IMB)))

# ---------------------------------------------------------------- host math

def _fused2x2(ph_first, ph_second):
    p = np.exp(1j * np.float64(ph_first))
    q = np.exp(1j * np.float64(ph_second))
    alpha = A * A * p - B * B * q
    beta = 1j * A * B * (p + q)
    delta = A * A * q - B * B * p
    return alpha, beta, delta


def _pack6(dst, aa, bb, dd):
    amb, dmb = aa - bb, dd - bb
    dst[:, 0] = bb.real
    dst[:, 1] = bb.imag
    dst[:, 2] = -bb.real
    dst[:, 3] = amb.real
    dst[:, 4] = amb.imag
    dst[:, 5] = -amb.real
    dst[:, 6] = dmb.real
    dst[:, 7] = dmb.imag
    dst[:, 8] = -dmb.real


def _precompute(phases, nsteps):
    ph = np.float64(phases)
    k = np.arange(256)
    j = np.arange(128)
    ceven = np.zeros((128, nsteps, 2, 9), np.float64)
    codd = np.zeros((128, nsteps, 2, 9), np.float64)
    for i in range(nsteps):
        pa = ph[1 + 2 * i]
        pb = ph[2 + 2 * i]
        al, be, de = _fused2x2(pa[2 * k], pa[2 * k + 1])
        for b in range(2):
            sel = 2 * j + b
            _pack6(ceven[:, i, b], al[sel], be[sel], de[sel])
        ko = np.arange(255)
        alo, beo, deo = _fused2x2(pb[2 * ko + 1], pb[2 * ko + 2])
        alo = np.concatenate([alo, [0.0 + 0j]])
        beo = np.concatenate([beo, [0.0 + 0j]])
        deo = np.concatenate([deo, [0.0 + 0j]])
        _pack6(codd[:, i, 0], alo[2 * j], beo[2 * j], deo[2 * j])
        sel1 = np.minimum(2 * j + 1, 255)
        a1, b1_, d1 = alo[sel1].copy(), beo[sel1].copy(), deo[sel1].copy()
        a1[127] = np.exp(1j * pb[511])   # row 511 rotation (u-channel)
        b1_[127] = 0.0
        d1[127] = np.exp(1j * pb[0])     # row 0 rotation (t-channel via Pbwd)
        _pack6(codd[:, i, 1], a1, b1_, d1)
    p_ = np.arange(128)
    cfin = np.zeros((128, 8), np.float64)
    phf = ph[N + 1]
    for b in range(2):
        rT = 2 * (2 * p_ + b)
        cfin[:, 0 + b] = np.cos(phf[rT])
        cfin[:, 2 + b] = np.sin(phf[rT])
        cfin[:, 4 + b] = np.cos(phf[rT + 1])
        cfin[:, 6 + b] = np.sin(phf[rT + 1])
    pfwd = np.zeros((128, 128), np.float32)
    pfwd[np.arange(1, 128), np.arange(0, 127)] = 1.0
    pfwd[0, 127] = 1.0
    pbwd = np.zeros((128, 128), np.float32)
    pbwd[np.arange(0, 127), np.arange(1, 128)] = 1.0
    pbwd[127, 0] = 1.0
    return (ceven.reshape(128, nsteps * 18).astype(np.float32),
            codd.reshape(128, nsteps * 18).astype(np.float32),
            cfin.astype(np.float32), pfwd, pbwd)


def _initial_state(phases, col0, ncols):
    """Packed [128, 8*ncols] init: channels Tre0,Tre1,Tim0,Tim1,Ure0..Uim1."""
    ph0 = np.float64(phases[0])
    out = np.zeros((128, 8, ncols), np.float64)
    p = np.arange(128)
    for b in range(2):
        kk = 2 * p + b
        rt = 2 * kk
        ru = rt + 1
        mt = (rt >= col0) & (rt < col0 + ncols)
        mu = (ru >= col0) & (ru < col0 + ncols)
        out[p[mt], 0 + b, rt[mt] - col0] = np.cos(ph0[rt[mt]])
        out[p[mt], 2 + b, rt[mt] - col0] = np.sin(ph0[rt[mt]])
        out[p[mu], 4 + b, ru[mu] - col0] = np.cos(ph0[ru[mu]])
        out[p[mu], 6 + b, ru[mu] - col0] = np.sin(ph0[ru[mu]])
    return out.reshape(128, 8 * ncols).astype(np.float32)

# ---------------------------------------------------------------- bass build

_CACHE = {}
_CMUL = []


def _ensure_cmul_op():
    """Register a custom DVE op: out = C0*Src0 - C1*Src1 (per-partition
    scalars). One uop; sha self-pinned at registration."""
    if _CMUL:
        return _CMUL[0]
    import concourse.dve_ops as D
    from concourse.dve_spec import Src0, Src1, C0, C1, lower, _has_src1
    from concourse.dve_uop import DveOpSpec
    from concourse.dve_table_gen import dve_ver_for

    name = "CMUL_SUB_ANT"
    for o in D.OPS:
        if o.name == name:
            _CMUL.append(o)
            return o
    spec = D.Spec(body=(Src0 * C0) - (Src1 * C1), accum=None, accum_init=None,
                  reference=lambda in0, in1, c0, c1, c2: in0 * c0 - in1 * c1)
    ver = dve_ver_for("TRN2")
    opcode = 1 + len(D.OPS)
    tmp = DveOpSpec(name=name, opcode=opcode, uops=lower(spec, ver=ver),
                    rd1_en=_has_src1(spec))
    op = D.DveOp(name=name, spec=spec, subdim=False,
                 uops_sha={ver: tmp.sha(ver)})
    D.OPS.append(op)
    D._SUB_OPCODE_FOR_NAME[name] = opcode
    D.CUSTOM_DVE_SPECS[name] = spec
    _CMUL.append(op)
    return op


def _build(nsteps=S):
    import concourse.mybir as mybir
    from concourse import bacc, tile

    f32 = mybir.dt.float32
    add, sub, mul = (mybir.AluOpType.add, mybir.AluOpType.subtract,
                     mybir.AluOpType.mult)

    nc = bacc.Bacc("TRN2", target_bir_lowering=False, debug=False,
                   enable_asserts=False)
    ce_d = nc.dram_tensor("ceven", [128, nsteps * 18], f32, kind="ExternalInput")
    co_d = nc.dram_tensor("codd", [128, nsteps * 18], f32, kind="ExternalInput")
    cf_d = nc.dram_tensor("cfin", [128, 8], f32, kind="ExternalInput")
    pf_d = nc.dram_tensor("pfwd", [128, 128], f32, kind="ExternalInput")
    pb_d = nc.dram_tensor("pbwd", [128, 128], f32, kind="ExternalInput")
    in_d = nc.dram_tensor("init", [128, 8 * COLS], f32, kind="ExternalInput")
    out_d = nc.dram_tensor("out", [128, 8 * COLS], f32, kind="ExternalOutput")

    with tile.TileContext(nc) as tc:
        with (
            tc.tile_pool(name="coef", bufs=1) as cpool,
            tc.tile_pool(name="state", bufs=4) as spool,
            tc.tile_pool(name="tmp", bufs=8) as tpool,
            tc.tile_pool(name="psum", bufs=2, space="PSUM") as ppool,
        ):
            ce = cpool.tile([128, nsteps * 18], f32, tag="ce")
            co = cpool.tile([128, nsteps * 18], f32, tag="co")
            cf = cpool.tile([128, 8], f32, tag="cf")
            pf = cpool.tile([128, 128], f32, tag="pf")
            pb = cpool.tile([128, 128], f32, tag="pb")
            ini = cpool.tile([128, 8 * COLS], f32, tag="ini")
            obuf = cpool.tile([128, 8 * COLS], f32, tag="obuf")
            nc.sync.dma_start(out=ce[:], in_=ce_d.ap())
            nc.sync.dma_start(out=co[:], in_=co_d.ap())
            nc.sync.dma_start(out=cf[:], in_=cf_d.ap())
            nc.sync.dma_start(out=pf[:], in_=pf_d.ap())
            nc.sync.dma_start(out=pb[:], in_=pb_d.ap())
            nc.sync.dma_start(out=ini[:], in_=in_d.ap())

            # current state APs per channel: Tre0,Tre1,Tim0,Tim1,Ure0,Ure1,Uim0,Uim1
            cur = [ini[:, ch * COLS:(ch + 1) * COLS] for ch in range(8)]

            cmul_op = _ensure_cmul_op()

            def cmul(out, i0, i1, sc0, sc1):
                # out = sc0*i0 - sc1*i1  (per-partition scalar columns)
                nc.vector._custom_dve(cmul_op, out=out, in0=i0, in1=i1,
                                      s0=sc0, s1=sc1)

            def half_block(tre, tim, ure, uim, coef, cb, outs,
                           bt=False, bu=False, s_on_dve=False):
                """Apply [[a,b],[b,d]] to (t,u); coef cols cb..cb+9 =
                br,bi,nbr, ar,ai,nar, dr,di,ndr (n* = negated).
                outs = (otre, otim, oure, ouim) destination APs.
                s-adds: GPSIMD tensor_tensor (DVE STT when a PSUM input).
                m and scheme-B rotations: one fused CMUL_SUB_ANT DVE op each;
                scheme-B final adds on GPSIMD. bt/bu pick scheme B for the
                t/u output pair; scheme A = 2 chained DVE STTs (PSUM-safe,
                shortest path for the PE-coupled slots)."""
                br = coef[:, cb + 0:cb + 1]
                bi = coef[:, cb + 1:cb + 2]
                nbr = coef[:, cb + 2:cb + 3]
                otre, otim, oure, ouim = outs
                v = nc.vector
                g = nc.gpsimd
                s_re = tpool.tile([128, COLS], f32, tag="s_re")
                s_im = tpool.tile([128, COLS], f32, tag="s_im")
                m_re = tpool.tile([128, COLS], f32, tag="m_re")
                m_im = tpool.tile([128, COLS], f32, tag="m_im")
                if s_on_dve:
                    v.scalar_tensor_tensor(out=s_re[:], in0=ure, scalar=1.0,
                                           in1=tre, op0=mul, op1=add)
                    v.scalar_tensor_tensor(out=s_im[:], in0=uim, scalar=1.0,
                                           in1=tim, op0=mul, op1=add)
                else:
                    g.tensor_add(out=s_re[:], in0=tre, in1=ure)
                    g.tensor_add(out=s_im[:], in0=tim, in1=uim)
                # m = beta * s (complex)
                cmul(m_re[:], s_re[:], s_im[:], br, bi)
                cmul(m_im[:], s_re[:], s_im[:], bi, nbr)

                def out_pair(ore, oim, xre, xim, c0, scheme_b):
                    # ore = cr*xre - ci*xim + m_re ; oim = ci*xre + cr*xim + m_im
                    cr = coef[:, cb + c0:cb + c0 + 1]
                    ci = coef[:, cb + c0 + 1:cb + c0 + 2]
                    ncr = coef[:, cb + c0 + 2:cb + c0 + 3]
                    if scheme_b:
                        z1 = tpool.tile([128, COLS], f32, tag="z1")
                        z2 = tpool.tile([128, COLS], f32, tag="z2")
                        cmul(z1[:], xre, xim, cr, ci)
                        g.tensor_add(out=ore, in0=z1[:], in1=m_re[:])
                        cmul(z2[:], xre, xim, ci, ncr)
                        g.tensor_add(out=oim, in0=z2[:], in1=m_im[:])
                    else:
                        v.scalar_tensor_tensor(out=ore, in0=xim, scalar=ci,
                                               in1=m_re[:], op0=mul, op1=sub)
                        v.scalar_tensor_tensor(out=ore, in0=xre, scalar=cr,
                                               in1=ore, op0=mul, op1=sub)
                        v.scalar_tensor_tensor(out=oim, in0=xre, scalar=ci,
                                               in1=m_im[:], op0=mul, op1=add)
                        v.scalar_tensor_tensor(out=oim, in0=xim, scalar=cr,
                                               in1=oim, op0=mul, op1=add)

                out_pair(otre, otim, tre, tim, 3, bt)
                out_pair(oure, ouim, ure, uim, 6, bu)

            for i in range(nsteps):
                # ---------------- even half ----------------
                nxt = [spool.tile([128, COLS], f32, tag=f"st{ch}", name=f"st{ch}_{i}")
                       for ch in range(8)]
                for b in range(2):
                    cb = (i * 2 + b) * 9
                    half_block(cur[0 + b], cur[2 + b], cur[4 + b], cur[6 + b],
                               ce, cb,
                               (nxt[0 + b][:], nxt[2 + b][:],
                                nxt[4 + b][:], nxt[6 + b][:]),
                               bt=False, bu=True, s_on_dve=(b == 0))
                # ---------------- odd half -----------------
                nx2 = [spool.tile([128, COLS], f32, tag=f"so{ch}", name=f"so{ch}_{i}")
                       for ch in range(8)]
                # range 0 (even k): (u = U[:,b0], t = T[:,b1]) aligned
                cb = (i * 2 + 0) * 9
                half_block(nxt[4][:], nxt[6][:], nxt[1][:], nxt[3][:],
                           co, cb,
                           (nx2[4][:], nx2[6][:], nx2[1][:], nx2[3][:]),
                           bt=True, bu=True)
                # PE shift: tsh = Pfwd . T'[:, b0]
                tsh_re = ppool.tile([128, COLS], f32, tag="tshre")
                tsh_im = ppool.tile([128, COLS], f32, tag="tshim")
                nc.tensor.matmul(out=tsh_re[:], lhsT=pf[:], rhs=nxt[0][:],
                                 start=True, stop=True)
                nc.tensor.matmul(out=tsh_im[:], lhsT=pf[:], rhs=nxt[2][:],
                                 start=True, stop=True)
                tshs_re = spool.tile([128, COLS], f32, tag="tshsre",
                                     name=f"tshsre_{i}")
                tshs_im = spool.tile([128, COLS], f32, tag="tshsim",
                                     name=f"tshsim_{i}")
                nc.scalar.copy(tshs_re[:], tsh_re[:])
                nc.scalar.copy(tshs_im[:], tsh_im[:])
                # range 1 (odd k): (u = U[:,b1], t = tsh)
                tt_re = tpool.tile([128, COLS], f32, tag="tt_re")
                tt_im = tpool.tile([128, COLS], f32, tag="tt_im")
                cb = (i * 2 + 1) * 9
                half_block(nxt[5][:], nxt[7][:], tshs_re[:], tshs_im[:],
                           co, cb,
                           (nx2[5][:], nx2[7][:], tt_re[:], tt_im[:]),
                           bt=True, bu=False, s_on_dve=True)
                # PE shift back: T''[:, b0] = Pbwd . tt  (lands in PSUM)
                t0_re = ppool.tile([128, COLS], f32, tag="t0re")
                t0_im = ppool.tile([128, COLS], f32, tag="t0im")
                nc.tensor.matmul(out=t0_re[:], lhsT=pb[:], rhs=tt_re[:],
                                 start=True, stop=True)
                nc.tensor.matmul(out=t0_im[:], lhsT=pb[:], rhs=tt_im[:],
                                 start=True, stop=True)
                cur = [t0_re[:], nx2[1][:], t0_im[:], nx2[3][:],
                       nx2[4][:], nx2[5][:], nx2[6][:], nx2[7][:]]

            # ---------------- final rotation + store ----------------
            v = nc.vector
            for tile_i in range(2):      # T, U
                for b in range(2):
                    cosc = cf[:, 4 * tile_i + b:4 * tile_i + b + 1]
                    sinc = cf[:, 4 * tile_i + 2 + b:4 * tile_i + 2 + b + 1]
                    re = cur[4 * tile_i + b]
                    im = cur[4 * tile_i + 2 + b]
                    ore = obuf[:, (4 * tile_i + b) * COLS:
                               (4 * tile_i + b + 1) * COLS]
                    oim = obuf[:, (4 * tile_i + 2 + b) * COLS:
                               (4 * tile_i + 2 + b + 1) * COLS]
                    x = tpool.tile([128, COLS], f32, tag="fx")
                    y = tpool.tile([128, COLS], f32, tag="fy")
                    v.tensor_scalar_mul(out=x[:], in0=im, scalar1=sinc)
                    v.scalar_tensor_tensor(out=ore, in0=re, scalar=cosc,
                                           in1=x[:], op0=mul, op1=sub)
                    v.tensor_scalar_mul(out=y[:], in0=re, scalar1=sinc)
                    v.scalar_tensor_tensor(out=oim, in0=im, scalar=cosc,
                                           in1=y[:], op0=mul, op1=add)
            nc.sync.dma_start(out=out_d.ap(), in_=obuf[:])
    nc.compile()
    return nc


def _get_module(nsteps=S):
    if nsteps not in _CACHE:
        _CACHE[nsteps] = _build(nsteps)
    return _CACHE[nsteps]


# ---------------------------------------------------------------- entry

def kernel(phases: np.ndarray) -> np.ndarray:
    from concourse.bass_utils import run_bass_kernel_spmd

    phases = np.asarray(phases)
    nc = _get_module(S)
    ce, co, cfin, pfwd, pbwd = _precompute(phases, S)
    in_maps = []
    for c in range(NCORES):
        in_maps.append({
            "ceven": ce, "codd": co, "cfin": cfin,
            "pfwd": pfwd, "pbwd": pbwd,
            "init": _initial_state(phases, c * COLS, COLS),
        })
    res = run_bass_kernel_spmd(nc, in_maps, core_ids=list(range(NCORES)))
    M = np.zeros((N, N), np.complex64)
    p = np.arange(128)
    for c in range(NCORES):
        o = res.results[c]["out"].reshape(128, 8, COLS)
        cols = slice(c * COLS, (c + 1) * COLS)
        for b in range(2):
            M[2 * (2 * p + b), cols] = o[:, 0 + b] + 1j * o[:, 2 + b]
            M[2 * (2 * p + b) + 1, cols] = o[:, 4 + b] + 1j * o[:, 6 + b]
    return M
